# revision 20
# baseline (speedup 1.0000x reference)
"""Trainium2 Bass kernel for nn_CandidateFinder (LSH hash-equality KNN).

Reference semantics: q/k binarized (x>0), projected by W [64,8], sign bits
packed into an 8-bit bucket code; for each query, return the first 64 key
indices (ascending) whose code equals the query's code, padded with -1.

Key insight: codes live in [0,256). Build, per batch, a [256, 64] table of
the first 64 key indices per bucket, then gather per query. Both steps map
onto matmuls + a free-dim prefix scan + GPSIMD local_scatters.

Sharding: 8 cores = 4 batches x 2 bucket-halves (c in [0,128) / [128,256)).
Each core computes a partial gather (zero where the query's code is in the
other half); host sums the pair and subtracts 1 (table stores j+1, empty=0).

Pipeline per core (engines balanced, quarter-granularity scan chain):
  k: bin(DVE) -> hash mm 4-chunk-stacked psum(PE) -> isgt 0/1(DVE)
     -> agree mm per chunk(PE) -> relu+bias(ACT) -> scan+mask(DVE)
     -> local_scatter(Pool) -> table merge(DVE)
  q: bin(Pool) -> hash mm(PE) -> sign +-1(ACT) -> agree mm(PE)
     -> one-hot: 2 chunks relu(ACT) + 2 chunks is_gt(DVE)
  out: gather mms(PE) -> psum copies (DVE+ACT) -> 2 DMAs (SP)

Precision: the hash sign test needs ~f32-accurate projections. W is split
as fp16(W) + fp16(W - fp16(W)) and the two fp16 matmuls accumulate in f32
PSUM; representation error ~1e-6 vs hash sign margins ~1e-4 on this data.

k-side agree trick on 0/1 bits: #agreeing bits = pm^T bits + (8 - pop(c))
with pm = +-1 bit pattern of bucket c, so onehot = Relu(pm^T bits + bias_c),
bias_c = 1 - pop(c). q-side agree on +-1 signs: onehot = Relu(pm^T s - 7).

Scan mask trick: scan init = -1024 so rank' = rank - 1024; scatter index
= onehot*1024 + rank' (one fused scalar_tensor_tensor) is rank>=1 at
matches and negative (ignored) elsewhere; table column 0 is garbage and
columns 1..64 hold the first 64 key indices (j+1) per bucket.
"""

import numpy as np
import ml_dtypes

B, L, D, NH = 4, 2048, 64, 8
KMAX = 64
TABLE_ELEMS = 256   # > max bucket count (90 on this data); idx beyond -> never
MPAD = 40           # hash matmul lhsT free size: 8 real + 32 zero rows
HALF = L // 2
QTR = L // 4

_cache = {}


def _build_program():
    import concourse.bass as bass
    import concourse.mybir as mybir
    from concourse import bacc, tile
    from contextlib import ExitStack

    dt = mybir.dt
    Alu = mybir.AluOpType
    Act = mybir.ActivationFunctionType

    nc = bacc.Bacc("TRN2", target_bir_lowering=False, debug=False)

    # DRAM I/O (per-core shapes)
    qT_d = nc.declare_dram_parameter("qT", [D, L], dt.bfloat16, isOutput=False)
    # kT half 0 with the fp16 [Whi | Wlo] weights (cols 8..MPAD of each
    # zero) packed as raw bytes in the last 2*MPAD columns: one DMA brings
    # both, so the hash matmuls are not gated on a separate weight load.
    kTw_d = nc.declare_dram_parameter("kTw", [D, HALF + 2 * MPAD], dt.bfloat16, isOutput=False)
    kT1_d = nc.declare_dram_parameter("kT1", [D, HALF], dt.bfloat16, isOutput=False)
    # pm (+-1 bit patterns) at partition rows 0..8 and 32..40
    sgnc_d = nc.declare_dram_parameter("sgnc", [D, 128], dt.float16, isOutput=False)
    biask_d = nc.declare_dram_parameter("biask", [128, 2], dt.float32, isOutput=False)
    out_d = nc.declare_dram_parameter("out", [L, KMAX], dt.float16, isOutput=True)

    with ExitStack() as ctx:
        tc = ctx.enter_context(tile.TileContext(nc))
        sb = ctx.enter_context(tc.tile_pool(name="sb", bufs=1))
        hp = ctx.enter_context(tc.tile_pool(name="hp", bufs=2, space="PSUM"))
        ap = ctx.enter_context(tc.tile_pool(name="ap", bufs=4, space="PSUM"))
        gp = ctx.enter_context(tc.tile_pool(name="gp", bufs=1, space="PSUM"))

        # ---- loads: kT+weights first (two queues), qT next, consts on ACT --
        kTw_sb = sb.tile([D, HALF + 2 * MPAD], dt.bfloat16, tag="kTw")
        nc.sync.dma_start(kTw_sb[:], kTw_d[:])
        kT1_sb = sb.tile([D, HALF], dt.bfloat16, tag="kT1")
        nc.gpsimd.dma_start(kT1_sb[:], kT1_d[:])
        wpk_sb = kTw_sb[:, HALF : HALF + 2 * MPAD].bitcast(dt.float16)
        qT_sb = sb.tile([D, L], dt.bfloat16, tag="qT")
        nc.sync.dma_start(qT_sb[:, 0:HALF], qT_d[:, 0:HALF])
        nc.gpsimd.dma_start(qT_sb[:, HALF:L], qT_d[:, HALF:L])
        sgnc_sb = sb.tile([D, 128], dt.float16, tag="sgnc")
        nc.scalar.dma_start(sgnc_sb[:], sgnc_d[:])
        biask_sb = sb.tile([128, 2], dt.float32, tag="biask")
        nc.scalar.dma_start(biask_sb[:], biask_d[:])

        # hash psum tiles: one [64, 512] tile per pair of chunks; chunk pair
        # (2g, 2g+1) lands at row offsets 0/32 of tile g. Even chunk uses
        # m=40 (8 real + 32 zero rows); the odd chunk's start=True then
        # overwrites rows 32..63 with its 8 real bits + 24 zeros, so all 64
        # rows are defined for the batched is_gt / Sign.
        hpk = [hp.tile([D, 512], dt.float32, tag="hp", name=f"hpk{g}") for g in range(2)]

        # PE warm-up: anchor the p-state clock (a >~3us idle resets the PE
        # ramp). Garbage results land in hpk[0] rows that the real hash
        # matmuls overwrite with start=True.
        warm_sb = sb.tile([D, 64], dt.float16, tag="warm")
        nc.vector.memset(warm_sb[:], 0.0)
        for _ in range(2):
            nc.tensor.matmul(
                hpk[0][0:32, 0:64], lhsT=warm_sb[:, 0:32], rhs=warm_sb[:],
                start=True, stop=True,
            )

        def hash_pair(hpt, x_sb, g):
            # chunks 2g (rows 0:40) and 2g+1 (rows 32:64) into tile hpt
            for i, (r, m) in enumerate(((0, MPAD), (32, 32))):
                u = 2 * g + i
                nc.tensor.matmul(
                    hpt[r : r + m, :],
                    lhsT=wpk_sb[:, 0:m], rhs=x_sb[:, 512 * u : 512 * (u + 1)],
                    start=True, stop=False,
                )
                nc.tensor.matmul(
                    hpt[r : r + m, :],
                    lhsT=wpk_sb[:, MPAD : MPAD + m],
                    rhs=x_sb[:, 512 * u : 512 * (u + 1)],
                    start=False, stop=True,
                )

        # ---- k side: bin (DVE) -> hash -> isgt 0/1 (DVE) -> agree ----
        xk = sb.tile([D, L], dt.float16, tag="xk")
        nc.vector.tensor_single_scalar(xk[:, 0:QTR], kTw_sb[:, 0:QTR], 0.0, Alu.is_gt)
        nc.vector.tensor_single_scalar(xk[:, QTR:HALF], kTw_sb[:, QTR:HALF], 0.0, Alu.is_gt)
        nc.vector.tensor_single_scalar(xk[:, HALF:L], kT1_sb[:], 0.0, Alu.is_gt)
        hash_pair(hpk[0], xk, 0)
        hash_pair(hpk[1], xk, 1)

        s01k = sb.tile([D, 1024], dt.float16, tag="s01k")
        nc.vector.tensor_single_scalar(s01k[:, 0:512], hpk[0][:], 0.0, Alu.is_gt)
        nc.vector.tensor_single_scalar(s01k[:, 512:1024], hpk[1][:], 0.0, Alu.is_gt)

        # ---- q side: bin (Pool) -> hash -> sign +-1 (ACT halves) -> agree --
        xq = sb.tile([D, L], dt.float16, tag="xq")
        nc.gpsimd.tensor_single_scalar(xq[:, 0:HALF], qT_sb[:, 0:HALF], 0.0, Alu.is_gt)
        nc.gpsimd.tensor_single_scalar(xq[:, HALF:L], qT_sb[:, HALF:L], 0.0, Alu.is_gt)

        # scatter data: each partition holds 0..L-1 (int16), off the
        # critical path (scat c0 needs it only after the first scan+mask)
        iota_sb = sb.tile([128, L], dt.float16, tag="iota")
        nc.gpsimd.iota(
            iota_sb[:], pattern=[[1, L]], base=1, channel_multiplier=0,
            allow_small_or_imprecise_dtypes=True,
        )

        onehot = sb.tile([128, L], dt.float16, tag="onehot")
        sq = sb.tile([D, 1024], dt.float16, tag="sq")
        q1h = sb.tile([128, 1536], dt.float16, tag="q1h")
        hpq = [hp.tile([D, 512], dt.float32, tag="hp", name=f"hpq{g}") for g in range(2)]

        def agree(u, rhs_sb, col, tag):
            r = 32 * (u % 2)
            g = u // 2
            t = ap.tile([128, 512], dt.float32, tag=tag, name=f"apt_{tag}_{u}")
            nc.tensor.matmul(
                t[:], lhsT=sgnc_sb[r : r + 8, :],
                rhs=rhs_sb[r : r + 8, 512 * g : 512 * (g + 1)],
                start=True, stop=True,
            )
            return t

        # PE emission order interleaves the q hash behind the k agrees so
        # the q chain keeps flowing while ACT works on the k relus.
        aptk = {}
        aptk[0] = agree(0, s01k, 0, "apt")
        aptk[1] = agree(1, s01k, 1, "apt")
        hash_pair(hpq[0], xq, 0)
        _unused = None
        aptk[2] = agree(2, s01k, 2, "apt")
        aptk[3] = agree(3, s01k, 3, "apt")
        hash_pair(hpq[1], xq, 1)

        for u in range(4):
            nc.scalar.activation(
                onehot[:, 512 * u : 512 * (u + 1)], aptk[u][:],
                Act.Relu, bias=biask_sb[:, 0:1],
            )
        nc.scalar.activation(sq[:, 0:512], hpq[0][:], Act.Sign)
        nc.scalar.activation(sq[:, 512:1024], hpq[1][:], Act.Sign)

        aptq = {}
        for u in range(4):
            aptq[u] = agree(u, sq, u, "apt")
        # q one-hot: chunks 0-2 on ACT, chunk 3 on DVE (after the scan chain)
        for u in range(3):
            nc.scalar.activation(
                q1h[:, 512 * u : 512 * (u + 1)], aptq[u][:],
                Act.Relu, bias=biask_sb[:, 1:2],
            )

        # ---- rank keys within bucket: quarter-granularity scan + mask.
        # m1 = onehot*rank is the 1-based rank at matches (0 elsewhere);
        # idx = m1 - 1 is the 0-based slot at matches, -1 (ignored) else.
        rank = sb.tile([128, L], dt.float16, tag="rank")
        m1 = sb.tile([128, L], dt.float16, tag="m1")
        idx16 = sb.tile([128, L], dt.int16, tag="idx16")
        tabs = []
        sub_inst = {}
        for c in range(4):
            lo, hi = QTR * c, QTR * (c + 1)
            init = 0.0 if c == 0 else rank[:, lo - 1 : lo]
            nc.vector.tensor_tensor_scan(
                rank[:, lo:hi], onehot[:, lo:hi], onehot[:, lo:hi],
                init, Alu.add, Alu.bypass,
            )
            nc.vector.tensor_mul(m1[:, lo:hi], onehot[:, lo:hi], rank[:, lo:hi])
            sub_inst[c] = nc.vector.tensor_single_scalar(
                idx16[:, lo:hi], m1[:, lo:hi], 1.0, Alu.subtract
            )
            tab = sb.tile([128, TABLE_ELEMS], dt.float16, tag=f"table{c}")
            tabs.append(tab)
            nc.gpsimd.local_scatter(
                tab[:], iota_sb[:, lo:hi], idx16[:, lo:hi],
                channels=128, num_elems=TABLE_ELEMS, num_idxs=QTR,
            )

        # q one-hot chunk 3 on DVE, held behind the scan chain so it does
        # not preempt the table build
        from concourse.tile_rust import add_dep_helper

        q1hx = sb.tile([128, 1024], dt.float16, tag="q1hx")
        aptq3_bf = aptq[3][:].bitcast(dt.bfloat16)
        q3_inst = nc.vector.tensor_single_scalar(q1hx[:], aptq3_bf, 7.0, Alu.is_gt)
        add_dep_helper(
            q3_inst.ins, sub_inst[3].ins, sync=False,
            reason="finish scan chain before q one-hot tail",
        )

        # merge quarter tables on Pool (disjoint nonzero slots); columns
        # 0..63 hold the first 64 matches (j+1) per bucket
        m01 = sb.tile([128, KMAX], dt.float16, tag="m01")
        nc.gpsimd.tensor_add(m01[:], tabs[0][:, 0:KMAX], tabs[1][:, 0:KMAX])
        m23 = sb.tile([128, KMAX], dt.float16, tag="m23")
        nc.gpsimd.tensor_add(m23[:], tabs[2][:, 0:KMAX], tabs[3][:, 0:KMAX])
        tab16 = sb.tile([128, KMAX], dt.float16, tag="tab16")
        nc.gpsimd.tensor_add(tab16[:], m01[:], m23[:])

        # ---- gather per query: out[i, s] = sum_c q1h[c, i] * tab16[c, s] ----
        # Chunk t takes queries {16p + t}, so psum partition p holds queries
        # 16p..16p+16 across chunks -> contiguous per-partition DRAM rows.
        q1hx_v = q1hx[:].rearrange("c (i two) -> c i two", two=2)[:, :, 1]
        HO = 8 * KMAX
        opA = gp.tile([128, HO], dt.float32, tag="gather", name="opA")
        opB = gp.tile([128, HO], dt.float32, tag="gatherB", name="opB")
        for t in range(16):
            dst = opA if t < 8 else opB
            if t < 12:
                lhsT = q1h[:, 128 * t : 128 * (t + 1)]
            else:
                lhsT = q1hx_v[:, 128 * (t - 12) : 128 * (t - 11)]
            nc.tensor.matmul(
                dst[:, KMAX * (t % 8) : KMAX * (t % 8 + 1)],
                lhsT=lhsT, rhs=tab16[:],
                start=True, stop=True,
            )
        out_v = out_d[:].rearrange("(p t) s -> p (t s)", p=128)  # [128, 1024] row-major view
        out0_sb = sb.tile([128, HO], dt.float16, tag="out0_sb")
        nc.vector.tensor_copy(out0_sb[:], opA[:])
        nc.sync.dma_start(out_v[:, 0:HO], out0_sb[:])
        out1_sb = sb.tile([128, HO], dt.float16, tag="out1_sb")
        nc.scalar.activation(out1_sb[:], opB[:], Act.Copy)
        nc.scalar.dma_start(out_v[:, HO : 2 * HO], out1_sb[:])

    nc.compile()
    return nc


def _get_nc():
    if "nc" not in _cache:
        _cache["nc"] = _build_program()
    return _cache["nc"]


def _make_in_maps(query, key, W):
    query = np.asarray(query, dtype=np.float32)
    key = np.asarray(key, dtype=np.float32)
    W = np.asarray(W, dtype=np.float32)
    qT = [
        np.ascontiguousarray(query[b].T).astype(ml_dtypes.bfloat16) for b in range(B)
    ]
    kT = [np.ascontiguousarray(key[b].T).astype(ml_dtypes.bfloat16) for b in range(B)]

    wpk = np.zeros((D, 2 * MPAD), np.float16)
    wpk[:, :NH] = W.astype(np.float16)
    wpk[:, MPAD : MPAD + NH] = (W - wpk[:, :NH].astype(np.float32)).astype(np.float16)
    wpk_as_bf16 = wpk.view(ml_dtypes.bfloat16)  # raw bytes, bitcast on device
    kTw = [
        np.ascontiguousarray(np.concatenate([kT[b][:, :HALF], wpk_as_bf16], axis=1))
        for b in range(B)
    ]
    kT1 = [np.ascontiguousarray(kT[b][:, HALF:]) for b in range(B)]

    sgnc = []
    biask = []
    for h in range(2):
        cg = 128 * h + np.arange(128)  # global bucket ids of this half
        bits = ((cg[None, :] >> np.arange(NH)[:, None]) & 1).astype(np.float32)
        pm = (2.0 * bits - 1.0).astype(np.float16)  # [8, 128]
        arr = np.zeros((D, 128), np.float16)
        arr[0:NH] = pm
        arr[32 : 32 + NH] = pm
        sgnc.append(arr)
        bk = np.empty((128, 2), np.float32)
        bk[:, 0] = 1.0 - bits.sum(axis=0)
        bk[:, 1] = -7.0
        biask.append(bk)
    return [
        {
            "qT": qT[c // 2],
            "kTw": kTw[c // 2],
            "kT1": kT1[c // 2],
            "sgnc": sgnc[c % 2],
            "biask": biask[c % 2],
        }
        for c in range(2 * B)
    ]


def _combine(results):
    # device layout: [128, 16*64], partition p col t*64+s <-> query 128t+p
    out = np.empty((B, L, KMAX), dtype=np.int64)
    for b in range(B):
        g = results[2 * b]["out"].astype(np.int64) + results[2 * b + 1]["out"].astype(
            np.int64
        )
        g = g.reshape(128, 16, KMAX).transpose(1, 0, 2).reshape(L, KMAX)
        out[b] = g - 1
    return out


def _run_spmd(in_maps, **kwargs):
    from concourse.bass_utils import run_bass_kernel_spmd

    return run_bass_kernel_spmd(_get_nc(), in_maps, list(range(2 * B)), **kwargs)


def kernel(query, key, W, head_idx=0, **_unused):
    in_maps = _make_in_maps(query, key, W)
    res = _run_spmd(in_maps)
    return _combine(res.results)
